# revision 16
# baseline (speedup 1.0000x reference)
"""Trainium2 Bass kernel for nn_AdditiveAttention (B=32, S=4096, D=512).

Data-parallel over batch: 8 NeuronCores x 4 batches each, no collectives.

Per batch b (all on device, per core):
    zT[h, s]  = sum_k W1kT[k, h] * KT[k, s]          (PE, bf16)
    hT        = tanh(zT + qb[b, h])                   (ACT, per-partition bias)
    score[s]  = sum_h W2[h] * hT[h, s]                (PE)
    attn      = softmax(score)                        (DVE/ACT, f32)
    applied   = attn @ V[b]                           (PE, bf16)
    out[b]    = applied @ WcaT + (x@WcxT + bc)        (PE + DVE)

Host staging (tiny, part of sharding prep): weight transposes + casts,
qbT = (Q @ W1q.T + b1).T and xc = x @ Wcx.T + bc  (0.03% of total FLOPs).
b2 is dropped: softmax is invariant to a per-row constant shift.
"""

import os
import sys
from contextlib import ExitStack

for _p in ("/opt/trn_rl_repo", "/root/.axon_site/_ro/trn_rl_repo"):
    if os.path.isdir(_p) and _p not in sys.path:
        sys.path.append(_p)

import numpy as np
import ml_dtypes

B, S, D = 32, 4096, 512
NCORES = 8
BL = B // NCORES  # 4 batches per core
KC = D // 128  # 4 contraction chunks
HC = D // 128  # 4 hidden chunks
SC = S // 512  # 8 s-chunks of 512
SS = 4  # 128-row subtiles per s-chunk

_CACHE = {}


def _build():
    import concourse.bass as bass
    import concourse.tile as tile
    from concourse import bacc, mybir

    f32 = mybir.dt.float32
    bf16 = mybir.dt.bfloat16
    ts = bass.ts

    nc = bacc.Bacc("TRN2", target_bir_lowering=False, debug=False,
                   num_devices=NCORES)

    kt_d = nc.dram_tensor("KT", [BL, D, S], f32, kind="ExternalInput").ap()
    v_d = nc.dram_tensor("V", [BL, S, D], f32, kind="ExternalInput").ap()
    w1kt_d = nc.dram_tensor("w1kt", [D, D], bf16, kind="ExternalInput").ap()
    w2c_d = nc.dram_tensor("w2c", [D, 1], bf16, kind="ExternalInput").ap()
    qbt_d = nc.dram_tensor("qbt", [D, BL], f32, kind="ExternalInput").ap()
    wcat_d = nc.dram_tensor("wcat", [D, D], bf16, kind="ExternalInput").ap()
    xc_d = nc.dram_tensor("xc", [BL, D], f32, kind="ExternalInput").ap()
    identb_d = nc.dram_tensor("identb", [128, 128], bf16, kind="ExternalInput").ap()
    identf_d = nc.dram_tensor("identf", [128, 128], f32, kind="ExternalInput").ap()

    out_d = nc.dram_tensor("out", [BL, D], f32, kind="ExternalOutput").ap()
    attn_d = nc.dram_tensor("attn", [BL, S], f32, kind="ExternalOutput").ap()

    Tanh = mybir.ActivationFunctionType.Tanh
    Exp = mybir.ActivationFunctionType.Exp
    AX = mybir.AxisListType.X
    MAX = mybir.AluOpType.max

    with tile.TileContext(nc) as tc, ExitStack() as ctx:
        wpool = ctx.enter_context(tc.tile_pool(name="weights", bufs=1))
        kin = ctx.enter_context(tc.tile_pool(name="kin", bufs=16))
        vin = ctx.enter_context(tc.tile_pool(name="vin", bufs=16))
        apool = ctx.enter_context(tc.tile_pool(name="apool", bufs=1, space="PSUM"))
        ktp = ctx.enter_context(tc.tile_pool(name="ktp", bufs=2, space="PSUM"))
        zps = ctx.enter_context(tc.tile_pool(name="zps", bufs=3, space="PSUM"))
        thp = ctx.enter_context(tc.tile_pool(name="thp", bufs=20))
        scps = ctx.enter_context(tc.tile_pool(name="scps", bufs=2, space="PSUM"))
        spool = ctx.enter_context(tc.tile_pool(name="spool", bufs=1))
        smalls = ctx.enter_context(tc.tile_pool(name="smalls", bufs=2))

        # ---- stage weights / constants ----
        w1kt_sb = wpool.tile([128, KC, D], bf16)
        nc.sync.dma_start(w1kt_sb[:], w1kt_d.rearrange("(c p) h -> p c h", p=128))
        w2c_sb = wpool.tile([128, HC], bf16)
        nc.sync.dma_start(w2c_sb[:], w2c_d.rearrange("(c p) o -> p (c o)", p=128))
        qbt_sb = wpool.tile([128, HC, BL], f32)
        nc.sync.dma_start(qbt_sb[:], qbt_d.rearrange("(c p) b -> p c b", p=128))
        wcat_sb = wpool.tile([128, KC, D], bf16)
        nc.sync.dma_start(wcat_sb[:], wcat_d.rearrange("(c p) h -> p c h", p=128))
        xc_sb = wpool.tile([BL, D], f32)
        nc.sync.dma_start(xc_sb[:], xc_d[:])
        identb = wpool.tile([128, 128], bf16)
        nc.sync.dma_start(identb[:], identb_d[:])
        identf = wpool.tile([128, 128], f32)
        nc.sync.dma_start(identf[:], identf_d[:])

        sum_parts = wpool.tile([128, SC], f32)
        score_sb = spool.tile([128, S], f32, tag="sp_a")
        nc.gpsimd.memset(score_sb[:], 0.0)
        p_bf = spool.tile([128, S], bf16, tag="sp_c")
        pT = wpool.tile([128, 32 * BL], bf16)
        app_ps = apool.tile([128, 512], f32)
        v_ins = {}

        # ---- P1 (sc-major): scores + exp + p-transposes, V prefetch ----
        for sc in range(SC):
            kt_ins = []
            for b in range(BL):
                kt_in = kin.tile([128, KC, 512], bf16, tag="kin")
                nc.gpsimd.dma_start(
                    kt_in[:],
                    kt_d[b, :, ts(sc, 512)].rearrange("(c p) s -> p c s", p=128),
                )
                kt_ins.append(kt_in)
            for b in range(BL):
                v_in = vin.tile([128, SS, 512], bf16, tag="vin")
                nc.gpsimd.dma_start(
                    v_in[:],
                    v_d[b, ts(sc, 512), :].rearrange("(a p) k -> p a k", p=128),
                )
                v_ins[(b, sc)] = v_in

            ths = {}
            for b in range(BL):
                for hc in range(HC):
                    z_ps = zps.tile([128, 512], f32, tag="zps")
                    for kc in range(KC):
                        nc.tensor.matmul(
                            z_ps[:],
                            w1kt_sb[:, kc, ts(hc, 128)],
                            kt_ins[b][:, kc, :],
                            start=(kc == 0),
                            stop=(kc == KC - 1),
                        )
                    th = thp.tile([128, 512], bf16, tag="thp")
                    nc.scalar.activation(
                        th[:], z_ps[:], Tanh, bias=qbt_sb[:, hc, b : b + 1]
                    )
                    ths[(b, hc)] = th

            score_ps = scps.tile([128, 512], f32, tag="scps")
            nc.vector.memset(score_ps[:], 0.0)
            for hc in range(HC):
                for b in range(BL):
                    nc.tensor.matmul(
                        score_ps[32 * b : 32 * b + 1, :],
                        w2c_sb[:, hc : hc + 1],
                        ths[(b, hc)][:],
                        start=(hc == 0),
                        stop=(hc == HC - 1),
                        tile_position=(0, 32 * b),
                        skip_group_check=True,
                    )
            nc.vector.tensor_copy(score_sb[:, ts(sc, 512)], score_ps[:])
            # exp + transposes of this column block, inside P1's shadow
            nc.scalar.activation(
                p_bf[:, ts(sc, 512)], score_sb[:, ts(sc, 512)], Exp
            )
            nc.vector.reduce_sum(
                sum_parts[:, sc : sc + 1], p_bf[:, ts(sc, 512)], axis=AX
            )
            for cc in range(4):
                c = sc * 4 + cc
                at_ps = ktp.tile([128, 128], bf16, tag="ktp")
                nc.tensor.transpose(at_ps[:], p_bf[:, ts(c, 128)], identb[:])
                nc.vector.tensor_copy(
                    pT[:, ts(c, BL)],
                    at_ps[:].rearrange("p (g r) -> p g r", r=32)[:, :, 0:1],
                )
            # online attn @ V for this chunk (no rescale: no max subtraction)
            for ss in range(SS):
                c = sc * SS + ss
                for b in range(BL):
                    nc.tensor.matmul(
                        app_ps[32 * b : 32 * b + 1, :],
                        pT[:, c * BL + b : c * BL + b + 1],
                        v_ins[(b, sc)][:, ss, :],
                        start=(c == 0),
                        stop=(c == 31),
                        tile_position=(0, 32 * b),
                        skip_group_check=True,
                    )

        # ---- P2: combine partial sums -> recip ----
        sums = smalls.tile([128, 1], f32, tag="sums")
        nc.vector.reduce_sum(sums[:], sum_parts[:], axis=AX)
        recip = smalls.tile([128, 1], f32, tag="recip")
        nc.vector.reciprocal(recip[:], sums[:])

        # ---- P3: scale applied by 1/sum ----
        app_sc = smalls.tile([128, 512], f32, tag="appsb")
        nc.vector.tensor_scalar_mul(app_sc[:], app_ps[:], recip[:])
        appT = wpool.tile([128, KC, BL], bf16)
        for dc in range(KC):
            apt_ps = scps.tile([128, 128], f32, tag="scps")
            nc.tensor.transpose(apt_ps[:], app_sc[:, ts(dc, 128)], identf[:])
            nc.vector.tensor_copy(
                appT[:, dc, :],
                apt_ps[:].rearrange("p (g r) -> p g r", r=32)[:, :, 0:1],
            )

        # ---- P4: out = appT.T @ WcaT + xc ----
        out_ps = zps.tile([BL, 512], f32, tag="zps")
        for dc in range(KC):
            nc.tensor.matmul(
                out_ps[:],
                appT[:, dc, :],
                wcat_sb[:, dc, :],
                start=(dc == 0),
                stop=(dc == KC - 1),
            )
        out_sb = smalls.tile([BL, 512], f32, tag="outsb")
        nc.vector.tensor_add(out_sb[:], out_ps[:], xc_sb[:])
        nc.sync.dma_start(out_d[:], out_sb[:])

        # normalized attn output (overlaps the epilogue)
        attn_n = spool.tile([128, S], f32, tag="sp_a2")
        nc.vector.tensor_scalar_mul(attn_n[:], p_bf[:], recip[:])
        for b in range(BL):
            nc.sync.dma_start(
                attn_d[b : b + 1, :], attn_n[32 * b : 32 * b + 1, :]
            )

    nc.compile()
    return nc


def _stage(inputs):
    """Host-side sharding + tiny weight preprocessing."""
    bf = ml_dtypes.bfloat16
    x = np.asarray(inputs["x"], np.float32)
    Q = np.asarray(inputs["Q"], np.float32)
    K = np.asarray(inputs["K"], np.float32)
    V = np.asarray(inputs["V"], np.float32)
    W1 = np.asarray(inputs["W1"], np.float32)
    b1 = np.asarray(inputs["b1"], np.float32)
    W2 = np.asarray(inputs["W2"], np.float32)
    Wc = np.asarray(inputs["Wc"], np.float32)
    bc = np.asarray(inputs["bc"], np.float32)

    W1q, W1k = W1[:, :D], W1[:, D:]
    Wcx, Wca = Wc[:, :D], Wc[:, D:]

    w1kt = np.ascontiguousarray(W1k.T).astype(bf)
    w2c = np.ascontiguousarray(W2.T).astype(bf)
    wcat = np.ascontiguousarray(Wca.T).astype(bf)
    identb = np.eye(128, dtype=bf)
    identf = np.eye(128, dtype=np.float32)

    qb = (Q[0].astype(np.float64) @ W1q.T.astype(np.float64)
          + b1.astype(np.float64))  # [B, D]
    xc = (x[:, 0].astype(np.float64) @ Wcx.T.astype(np.float64)
          + bc.astype(np.float64))  # [B, D]

    in_maps = []
    for i in range(NCORES):
        sl = slice(BL * i, BL * (i + 1))
        in_maps.append({
            "KT": np.ascontiguousarray(K[sl].transpose(0, 2, 1)),
            "V": np.ascontiguousarray(V[sl]),
            "w1kt": w1kt,
            "w2c": w2c,
            "qbt": np.ascontiguousarray(qb[sl].T).astype(np.float32),
            "wcat": wcat,
            "xc": xc[sl].astype(np.float32),
            "identb": identb,
            "identf": identf,
        })
    return in_maps


def _install_profile_hook():
    import types

    if "antenv.axon_hooks" not in sys.modules:
        mod = types.ModuleType("antenv.axon_hooks")
        _state = {"hook": None}
        mod.set_axon_ntff_profile_hook = lambda h: _state.__setitem__("hook", h)
        mod.get_axon_ntff_profile_hook = lambda: _state["hook"]
        sys.modules["antenv.axon_hooks"] = mod
        try:
            import antenv
            antenv.axon_hooks = mod
        except ImportError:
            pass
    from antenv.axon_hooks import (
        get_axon_ntff_profile_hook,
        set_axon_ntff_profile_hook,
    )
    if get_axon_ntff_profile_hook() is None:
        from trn_agent_boot.trn_boot import _ntff_profile_via_ctypes
        set_axon_ntff_profile_hook(
            _ntff_profile_via_ctypes("/opt/axon/libaxon_pjrt.so"))
    import concourse.bass_utils as bu
    bu.upload_artifacts = lambda tmpdir: f"local:{tmpdir}"


def _run(inputs, trace=False):
    from concourse.bass_utils import run_bass_kernel_spmd

    if trace:
        _install_profile_hook()
    if "nc" not in _CACHE:
        _CACHE["nc"] = _build()
    nc = _CACHE["nc"]
    in_maps = _stage(inputs)
    res = run_bass_kernel_spmd(nc, in_maps, list(range(NCORES)), trace=trace)
    out = np.stack([res.results[i]["out"] for i in range(NCORES)])
    attn = np.stack([res.results[i]["attn"] for i in range(NCORES)])
    out_full = out.reshape(B, 1, D).astype(np.float32)
    attn_full = attn.reshape(B, 1, S).astype(np.float32)
    return (out_full, attn_full), res


def kernel(**inputs):
    (out_full, attn_full), _ = _run(inputs, trace=False)
    return out_full, attn_full


# revision 17
# speedup vs baseline: 1.1463x; 1.1463x over previous
"""Trainium2 Bass kernel for nn_AdditiveAttention (B=32, S=4096, D=512).

Data-parallel over batch: 8 NeuronCores x 4 batches each, no collectives.

Per batch b (all on device, per core):
    zT[h, s]  = sum_k W1kT[k, h] * KT[k, s]          (PE, bf16)
    hT        = tanh(zT + qb[b, h])                   (ACT, per-partition bias)
    score[s]  = sum_h W2[h] * hT[h, s]                (PE)
    attn      = softmax(score)                        (DVE/ACT, f32)
    applied   = attn @ V[b]                           (PE, bf16)
    out[b]    = applied @ WcaT + (x@WcxT + bc)        (PE + DVE)

Host staging (tiny, part of sharding prep): weight transposes + casts,
qbT = (Q @ W1q.T + b1).T and xc = x @ Wcx.T + bc  (0.03% of total FLOPs).
b2 is dropped: softmax is invariant to a per-row constant shift.
"""

import os
import sys
from contextlib import ExitStack

for _p in ("/opt/trn_rl_repo", "/root/.axon_site/_ro/trn_rl_repo"):
    if os.path.isdir(_p) and _p not in sys.path:
        sys.path.append(_p)

import numpy as np
import ml_dtypes

B, S, D = 32, 4096, 512
NCORES = 8
BL = B // NCORES  # 4 batches per core
KC = D // 128  # 4 contraction chunks
HC = D // 128  # 4 hidden chunks
SC = S // 512  # 8 s-chunks of 512
SS = 4  # 128-row subtiles per s-chunk

_CACHE = {}


def _build():
    import concourse.bass as bass
    import concourse.tile as tile
    from concourse import bacc, mybir

    f32 = mybir.dt.float32
    bf16 = mybir.dt.bfloat16
    ts = bass.ts

    nc = bacc.Bacc("TRN2", target_bir_lowering=False, debug=False,
                   num_devices=NCORES)

    kt_d = nc.dram_tensor("KT", [BL, D, S], f32, kind="ExternalInput").ap()
    v_d = nc.dram_tensor("V", [BL, S, D], f32, kind="ExternalInput").ap()
    w1kt_d = nc.dram_tensor("w1kt", [D, D], bf16, kind="ExternalInput").ap()
    w2c_d = nc.dram_tensor("w2c", [D, 1], bf16, kind="ExternalInput").ap()
    qbt_d = nc.dram_tensor("qbt", [D, BL], f32, kind="ExternalInput").ap()
    wcat_d = nc.dram_tensor("wcat", [D, D], bf16, kind="ExternalInput").ap()
    xc_d = nc.dram_tensor("xc", [BL, D], f32, kind="ExternalInput").ap()
    identb_d = nc.dram_tensor("identb", [128, 128], bf16, kind="ExternalInput").ap()
    identf_d = nc.dram_tensor("identf", [128, 128], f32, kind="ExternalInput").ap()

    out_d = nc.dram_tensor("out", [BL, D], f32, kind="ExternalOutput").ap()
    attn_d = nc.dram_tensor("attn", [BL, S], f32, kind="ExternalOutput").ap()

    Tanh = mybir.ActivationFunctionType.Tanh
    Exp = mybir.ActivationFunctionType.Exp
    AX = mybir.AxisListType.X
    MAX = mybir.AluOpType.max

    with tile.TileContext(nc) as tc, ExitStack() as ctx:
        wpool = ctx.enter_context(tc.tile_pool(name="weights", bufs=1))
        kin = ctx.enter_context(tc.tile_pool(name="kin", bufs=16))
        vin = ctx.enter_context(tc.tile_pool(name="vin", bufs=16))
        apool = ctx.enter_context(tc.tile_pool(name="apool", bufs=1, space="PSUM"))
        ktp = ctx.enter_context(tc.tile_pool(name="ktp", bufs=2, space="PSUM"))
        zps = ctx.enter_context(tc.tile_pool(name="zps", bufs=3, space="PSUM"))
        thp = ctx.enter_context(tc.tile_pool(name="thp", bufs=20))
        scps = ctx.enter_context(tc.tile_pool(name="scps", bufs=2, space="PSUM"))
        spool = ctx.enter_context(tc.tile_pool(name="spool", bufs=1))
        smalls = ctx.enter_context(tc.tile_pool(name="smalls", bufs=2))

        # ---- stage weights / constants ----
        w1kt_sb = wpool.tile([128, KC, D], bf16)
        nc.sync.dma_start(w1kt_sb[:], w1kt_d.rearrange("(c p) h -> p c h", p=128))
        w2c_sb = wpool.tile([128, HC], bf16)
        nc.sync.dma_start(w2c_sb[:], w2c_d.rearrange("(c p) o -> p (c o)", p=128))
        qbt_sb = wpool.tile([128, HC, BL], f32)
        nc.sync.dma_start(qbt_sb[:], qbt_d.rearrange("(c p) b -> p c b", p=128))
        wcat_sb = wpool.tile([128, KC, D], bf16)
        nc.sync.dma_start(wcat_sb[:], wcat_d.rearrange("(c p) h -> p c h", p=128))
        xc_sb = wpool.tile([BL, D], f32)
        nc.sync.dma_start(xc_sb[:], xc_d[:])
        identb = wpool.tile([128, 128], bf16)
        nc.sync.dma_start(identb[:], identb_d[:])
        identf = wpool.tile([128, 128], f32)
        nc.sync.dma_start(identf[:], identf_d[:])

        sum_parts = wpool.tile([128, SC + 1], f32)
        score_sb = spool.tile([128, S], f32, tag="sp_a")
        nc.gpsimd.memset(score_sb[:], 0.0)
        p_bf = spool.tile([128, S], bf16, tag="sp_c")
        pT = wpool.tile([128, 32 * BL], bf16)
        app_ps = apool.tile([128, 512], f32)
        v_ins = {}
        chunks = [(i * 512, 512) for i in range(7)] + [(3584, 256), (3840, 256)]

        # ---- P1 (chunk-major): scores + exp + p-transposes, V prefetch ----
        for ci, (off, csz) in enumerate(chunks):
            kt_ins = []
            for b in range(BL):
                kt_in = kin.tile([128, KC, csz], bf16, tag="kin")
                nc.gpsimd.dma_start(
                    kt_in[:],
                    kt_d[b, :, off : off + csz].rearrange(
                        "(c p) s -> p c s", p=128
                    ),
                )
                kt_ins.append(kt_in)
            for b in range(BL):
                v_in = vin.tile([128, csz // 128, 512], bf16, tag="vin")
                nc.gpsimd.dma_start(
                    v_in[:],
                    v_d[b, off : off + csz, :].rearrange(
                        "(a p) k -> p a k", p=128
                    ),
                )
                v_ins[(b, ci)] = v_in

            ths = {}
            for b in range(BL):
                for hc in range(HC):
                    z_ps = zps.tile([128, csz], f32, tag="zps")
                    for kc in range(KC):
                        nc.tensor.matmul(
                            z_ps[:],
                            w1kt_sb[:, kc, ts(hc, 128)],
                            kt_ins[b][:, kc, :],
                            start=(kc == 0),
                            stop=(kc == KC - 1),
                        )
                    th = thp.tile([128, csz], bf16, tag="thp")
                    nc.scalar.activation(
                        th[:], z_ps[:], Tanh, bias=qbt_sb[:, hc, b : b + 1]
                    )
                    ths[(b, hc)] = th

            score_ps = scps.tile([128, csz], f32, tag="scps")
            nc.vector.memset(score_ps[:], 0.0)
            for hc in range(HC):
                for b in range(BL):
                    nc.tensor.matmul(
                        score_ps[32 * b : 32 * b + 1, :],
                        w2c_sb[:, hc : hc + 1],
                        ths[(b, hc)][:],
                        start=(hc == 0),
                        stop=(hc == HC - 1),
                        tile_position=(0, 32 * b),
                        skip_group_check=True,
                    )
            nc.vector.tensor_copy(score_sb[:, off : off + csz], score_ps[:])
            # exp + transposes of this column block, inside P1's shadow
            nc.scalar.activation(
                p_bf[:, off : off + csz], score_sb[:, off : off + csz], Exp
            )
            nc.vector.reduce_sum(
                sum_parts[:, ci : ci + 1], p_bf[:, off : off + csz], axis=AX
            )
            for cc in range(csz // 128):
                c = off // 128 + cc
                at_ps = ktp.tile([128, 128], bf16, tag="ktp")
                nc.tensor.transpose(at_ps[:], p_bf[:, ts(c, 128)], identb[:])
                nc.vector.tensor_copy(
                    pT[:, ts(c, BL)],
                    at_ps[:].rearrange("p (g r) -> p g r", r=32)[:, :, 0:1],
                )
            # online attn @ V for this chunk (no rescale: no max subtraction)
            for ss in range(csz // 128):
                c = off // 128 + ss
                for b in range(BL):
                    nc.tensor.matmul(
                        app_ps[32 * b : 32 * b + 1, :],
                        pT[:, c * BL + b : c * BL + b + 1],
                        v_ins[(b, ci)][:, ss, :],
                        start=(c == 0),
                        stop=(c == 31),
                        tile_position=(0, 32 * b),
                        skip_group_check=True,
                    )

        # ---- P2: combine partial sums -> recip ----
        sums = smalls.tile([128, 1], f32, tag="sums")
        nc.vector.reduce_sum(sums[:], sum_parts[:], axis=AX)
        recip = smalls.tile([128, 1], f32, tag="recip")
        nc.vector.reciprocal(recip[:], sums[:])

        # ---- P3: scale applied by 1/sum ----
        app_sc = smalls.tile([128, 512], f32, tag="appsb")
        nc.vector.tensor_scalar_mul(app_sc[:], app_ps[:], recip[:])
        appT = wpool.tile([128, KC, BL], bf16)
        for dc in range(KC):
            apt_ps = scps.tile([128, 128], f32, tag="scps")
            nc.tensor.transpose(apt_ps[:], app_sc[:, ts(dc, 128)], identf[:])
            nc.vector.tensor_copy(
                appT[:, dc, :],
                apt_ps[:].rearrange("p (g r) -> p g r", r=32)[:, :, 0:1],
            )

        # ---- P4: out = appT.T @ WcaT + xc ----
        out_ps = zps.tile([BL, 512], f32, tag="zps")
        for dc in range(KC):
            nc.tensor.matmul(
                out_ps[:],
                appT[:, dc, :],
                wcat_sb[:, dc, :],
                start=(dc == 0),
                stop=(dc == KC - 1),
            )
        out_sb = smalls.tile([BL, 512], f32, tag="outsb")
        nc.vector.tensor_add(out_sb[:], out_ps[:], xc_sb[:])
        nc.sync.dma_start(out_d[:], out_sb[:])

        # normalized attn output (overlaps the epilogue)
        attn_n = spool.tile([128, S], f32, tag="sp_a2")
        nc.vector.tensor_scalar_mul(attn_n[:], p_bf[:], recip[:])
        for b in range(BL):
            nc.sync.dma_start(
                attn_d[b : b + 1, :], attn_n[32 * b : 32 * b + 1, :]
            )

    nc.compile()
    return nc


def _stage(inputs):
    """Host-side sharding + tiny weight preprocessing."""
    bf = ml_dtypes.bfloat16
    x = np.asarray(inputs["x"], np.float32)
    Q = np.asarray(inputs["Q"], np.float32)
    K = np.asarray(inputs["K"], np.float32)
    V = np.asarray(inputs["V"], np.float32)
    W1 = np.asarray(inputs["W1"], np.float32)
    b1 = np.asarray(inputs["b1"], np.float32)
    W2 = np.asarray(inputs["W2"], np.float32)
    Wc = np.asarray(inputs["Wc"], np.float32)
    bc = np.asarray(inputs["bc"], np.float32)

    W1q, W1k = W1[:, :D], W1[:, D:]
    Wcx, Wca = Wc[:, :D], Wc[:, D:]

    w1kt = np.ascontiguousarray(W1k.T).astype(bf)
    w2c = np.ascontiguousarray(W2.T).astype(bf)
    wcat = np.ascontiguousarray(Wca.T).astype(bf)
    identb = np.eye(128, dtype=bf)
    identf = np.eye(128, dtype=np.float32)

    qb = (Q[0].astype(np.float64) @ W1q.T.astype(np.float64)
          + b1.astype(np.float64))  # [B, D]
    xc = (x[:, 0].astype(np.float64) @ Wcx.T.astype(np.float64)
          + bc.astype(np.float64))  # [B, D]

    in_maps = []
    for i in range(NCORES):
        sl = slice(BL * i, BL * (i + 1))
        in_maps.append({
            "KT": np.ascontiguousarray(K[sl].transpose(0, 2, 1)),
            "V": np.ascontiguousarray(V[sl]),
            "w1kt": w1kt,
            "w2c": w2c,
            "qbt": np.ascontiguousarray(qb[sl].T).astype(np.float32),
            "wcat": wcat,
            "xc": xc[sl].astype(np.float32),
            "identb": identb,
            "identf": identf,
        })
    return in_maps


def _install_profile_hook():
    import types

    if "antenv.axon_hooks" not in sys.modules:
        mod = types.ModuleType("antenv.axon_hooks")
        _state = {"hook": None}
        mod.set_axon_ntff_profile_hook = lambda h: _state.__setitem__("hook", h)
        mod.get_axon_ntff_profile_hook = lambda: _state["hook"]
        sys.modules["antenv.axon_hooks"] = mod
        try:
            import antenv
            antenv.axon_hooks = mod
        except ImportError:
            pass
    from antenv.axon_hooks import (
        get_axon_ntff_profile_hook,
        set_axon_ntff_profile_hook,
    )
    if get_axon_ntff_profile_hook() is None:
        from trn_agent_boot.trn_boot import _ntff_profile_via_ctypes
        set_axon_ntff_profile_hook(
            _ntff_profile_via_ctypes("/opt/axon/libaxon_pjrt.so"))
    import concourse.bass_utils as bu
    bu.upload_artifacts = lambda tmpdir: f"local:{tmpdir}"


def _run(inputs, trace=False):
    from concourse.bass_utils import run_bass_kernel_spmd

    if trace:
        _install_profile_hook()
    if "nc" not in _CACHE:
        _CACHE["nc"] = _build()
    nc = _CACHE["nc"]
    in_maps = _stage(inputs)
    res = run_bass_kernel_spmd(nc, in_maps, list(range(NCORES)), trace=trace)
    out = np.stack([res.results[i]["out"] for i in range(NCORES)])
    attn = np.stack([res.results[i]["attn"] for i in range(NCORES)])
    out_full = out.reshape(B, 1, D).astype(np.float32)
    attn_full = attn.reshape(B, 1, S).astype(np.float32)
    return (out_full, attn_full), res


def kernel(**inputs):
    (out_full, attn_full), _ = _run(inputs, trace=False)
    return out_full, attn_full
